# revision 11
# baseline (speedup 1.0000x reference)
"""GroupQuantizedLinear Trainium2 kernel.

y = x @ dequant(weights, scales).T, split at 14336.
  x: [2048, 4096] f32, weights: [28672, 4096] f32, scales: [28672, 32] f32
  dequant: round(clip(w,-8,7)) * group_scale (group=128 along input dim)

Sharding: column-parallel — each of 8 cores gets 3584 output channels
(weights/scales rows); x replicated. Core outputs [2048, 3584] are
concatenated on host, then split into the (14336, 14336) tuple.

Per-core device kernel (all transposes via PE identity-matmul, bf16):
  phase X: x -> SBUF, convert bf16, transpose to xT [i,t] resident slab
  per 128-row o-tile:
    W fp32 -> RNE round ((w+C)-C, C=3*2^22) -> per-group scale -> bf16
    transpose to wT [i,o]; 32x accumulate matmuls -> PSUM [o, t512]
    PSUM -> bf16, transpose back to [t, o], store fp32
"""

import sys

if "/opt/trn_rl_repo" not in sys.path:
    sys.path.insert(0, "/opt/trn_rl_repo")

import numpy as np
import ml_dtypes

import concourse.bass as bass
import concourse.bacc as bacc
import concourse.tile as tile
from concourse import mybir
from concourse.bass_utils import run_bass_kernel_spmd

N_CORES = 8
T = 2048          # tokens
I = 4096          # in features
O_TOT = 28672     # total out features
O_SH = O_TOT // N_CORES   # 3584 per core
G = 32            # scale groups (of 128) along I
SPLIT = 14336

NK = I // 128     # 32 contraction chunks
NT = T // 128     # 16 token tiles
NO = O_SH // 128  # 28 out tiles per core
RC = float(3 * 2**22)  # 12582912.0 — RNE round-to-int bias for |w| < 2^22

F32 = mybir.dt.float32
BF16 = mybir.dt.bfloat16
ADD = mybir.AluOpType.add
SUB = mybir.AluOpType.subtract
MUL = mybir.AluOpType.mult

_CACHE = {}


def build_nc(t=T, o_sh=O_SH):
    nt = t // 128
    ntc = max(t // 512, 1)
    tcw = min(t, 512)          # token chunk width for matmul free dim
    no = o_sh // 128

    nc = bacc.Bacc(
        "TRN2", target_bir_lowering=False, debug=False, num_devices=N_CORES
    )
    x_d = nc.dram_tensor("x", (t, I), F32, kind="ExternalInput")
    w_d = nc.dram_tensor("w", (o_sh, I), F32, kind="ExternalInput")
    s_d = nc.dram_tensor("s", (o_sh, G), F32, kind="ExternalInput")
    e_d = nc.dram_tensor("ident", (128, 128), BF16, kind="ExternalInput")
    # Output is y.T per core ([o_sh, t]) — contiguous 8KB DMA lines; the
    # final transpose happens on the host during shard assembly, saving
    # ~0.4 ms/core of PE transposes + copies on the device.
    y_d = nc.dram_tensor("y", (o_sh, t), F32, kind="ExternalOutput")

    with tile.TileContext(nc) as tc:
        with (
            tc.tile_pool(name="consts", bufs=1) as consts,
            tc.tile_pool(name="raw", bufs=2) as raw,
            tc.tile_pool(name="h16", bufs=2) as h16,
            tc.tile_pool(name="wTp", bufs=2) as wTp,
            tc.tile_pool(name="xTp", bufs=1) as xTp,
            tc.tile_pool(name="y16p", bufs=2) as y16p,
            tc.tile_pool(name="sclp", bufs=2) as sclp,
            tc.tile_pool(name="ps_t", bufs=4, space=bass.MemorySpace.PSUM) as ps_t,
            tc.tile_pool(name="ps_a", bufs=4, space=bass.MemorySpace.PSUM) as ps_a,
        ):
            ident = consts.tile([128, 128], BF16)
            nc.gpsimd.dma_start(ident[:], e_d[:])

            # Resident transposed activations: xT[:, k*t + tt] = x[tt, k*128+p]
            xT = xTp.tile([128, NK * t], BF16)

            for tt in range(nt):
                for h in range(2):
                    xr = raw.tile([128, 2048], F32, tag="stage")
                    nc.gpsimd.dma_start(
                        xr[:], x_d[tt * 128:(tt + 1) * 128, h * 2048:(h + 1) * 2048]
                    )
                    xb = h16.tile([128, 2048], BF16, tag="stage16")
                    nc.vector.tensor_copy(xb[:], xr[:])
                    for kk in range(16):
                        k = h * 16 + kk
                        pt = ps_t.tile([128, 128], F32)
                        nc.tensor.matmul(
                            pt[:], xb[:, kk * 128:(kk + 1) * 128], ident[:],
                            start=True, stop=True,
                        )
                        nc.vector.tensor_copy(
                            xT[:, k * t + tt * 128: k * t + (tt + 1) * 128], pt[:]
                        )

            for ot in range(no):
                ssb = sclp.tile([128, G], F32)
                nc.gpsimd.dma_start(ssb[:], s_d[ot * 128:(ot + 1) * 128, :])
                wT = wTp.tile([128, NK * 128], BF16)
                for h in range(2):
                    wr = raw.tile([128, 2048], F32, tag="stage")
                    nc.gpsimd.dma_start(
                        wr[:], w_d[ot * 128:(ot + 1) * 128, h * 2048:(h + 1) * 2048]
                    )
                    # RNE round to integer grid, in place
                    nc.vector.tensor_scalar(wr[:], wr[:], RC, RC, ADD, SUB)
                    wq = h16.tile([128, 2048], BF16, tag="stage16")
                    for gg in range(16):
                        g = h * 16 + gg
                        nc.vector.tensor_scalar(
                            wq[:, gg * 128:(gg + 1) * 128],
                            wr[:, gg * 128:(gg + 1) * 128],
                            ssb[:, g:g + 1], None, MUL,
                        )
                    for gg in range(16):
                        k = h * 16 + gg
                        pt = ps_t.tile([128, 128], F32)
                        nc.tensor.matmul(
                            pt[:], wq[:, gg * 128:(gg + 1) * 128], ident[:],
                            start=True, stop=True,
                        )
                        nc.scalar.copy(wT[:, k * 128:(k + 1) * 128], pt[:])

                accs = [
                    ps_a.tile([128, tcw], F32, tag="acc", name=f"acc{ci}")
                    for ci in range(ntc)
                ]
                for k in range(NK):
                    for ci in range(ntc):
                        nc.tensor.matmul(
                            accs[ci][:],
                            wT[:, k * 128:(k + 1) * 128],
                            xT[:, k * t + ci * tcw: k * t + (ci + 1) * tcw],
                            start=(k == 0), stop=(k == NK - 1),
                        )
                yf = y16p.tile([128, t], F32, tag="yf")
                for ci in range(ntc):
                    nc.vector.tensor_copy(yf[:, ci * tcw:(ci + 1) * tcw], accs[ci][:])
                nc.gpsimd.dma_start(y_d[ot * 128:(ot + 1) * 128, :], yf[:])

    nc.compile()
    return nc


def _get_nc():
    if "nc" not in _CACHE:
        _CACHE["nc"] = build_nc()
    return _CACHE["nc"]


def _run(x, weights, scales, trace=False):
    x = np.ascontiguousarray(np.asarray(x, dtype=np.float32))
    weights = np.ascontiguousarray(np.asarray(weights, dtype=np.float32))
    scales = np.ascontiguousarray(np.asarray(scales, dtype=np.float32))
    ident = np.eye(128, dtype=ml_dtypes.bfloat16)

    in_maps = []
    for c in range(N_CORES):
        sl = slice(c * O_SH, (c + 1) * O_SH)
        in_maps.append({
            "x": x,
            "w": np.ascontiguousarray(weights[sl]),
            "s": np.ascontiguousarray(scales[sl]),
            "ident": ident,
        })
    br = run_bass_kernel_spmd(_get_nc(), in_maps, list(range(N_CORES)), trace=trace)
    # Cores return y.T shards [O_SH, T]; stack and transpose on host.
    yt = np.concatenate([br.results[c]["y"] for c in range(N_CORES)], axis=0)
    y = np.ascontiguousarray(yt.T)
    return y, br


def kernel(x, weights, scales):
    y, _ = _run(x, weights, scales, trace=False)
    return tuple(np.split(y, [SPLIT], axis=-1))
